# revision 5
# baseline (speedup 1.0000x reference)
"""Trainium2 Bass kernel for nn_EnhancedMoMGRUCell.

Data-parallel over batch: 8 NeuronCores x 128 rows each (= SBUF partition
count). Per core, batch rows live on partitions; the per-sample attention
over memory banks runs on DVE/ACT/GPSIMD; dense GRU/router matmuls run on
the PE in bf16 with activations stationary and weights moving (N=512).

Memory-bank tensors use an (m, d, s) on-chip layout so every broadcast
multiply has a unit-stride innermost axis (DVE 2x bf16 mode); reductions
over d and s are log-depth tensor_add trees in bf16 with fp32 final level.
The final memory update (mem + outer) is fp32.

Shapes (full): x(1024,512) h(1024,1024) memory_state(1024,4,128,64).
"""
import numpy as np
import ml_dtypes

import concourse.bass as bass
import concourse.mybir as mybir
import concourse.tile as tile
import bass_rust
from concourse.bass_utils import run_bass_kernel_spmd
from concourse.masks import make_identity

B, IN, H, M, S, D, RH = 1024, 512, 1024, 4, 128, 64, 32
NCORES = 8
BL = B // NCORES          # 128 batch rows per core
KRIN = 13                 # ceil((IN+H+1)/128): rin + ones row, padded
KXA = 5                   # ceil((IN+D+1)/128): xa + bias row, padded
KH = 8                    # H/128
KKVG = 9                  # H/128 + 1 bias tile
F32 = mybir.dt.float32
BF16 = mybir.dt.bfloat16
AX = mybir.AxisListType
AF = mybir.ActivationFunctionType
BF_NP = ml_dtypes.bfloat16

_CACHE = {}


def _split_excess_waits(nc, max_waits=1):
    """This walrus build rejects instructions with >1 sync-wait; move excess
    waits onto preceding same-engine NoOps (semantically identical)."""
    k = 0
    for f in nc.m.functions:
        for bb in f.blocks:
            insts = bb.instructions
            new_list = []
            for inst in insts:
                si = inst.sync_info
                if si is not None and si.on_wait and len(si.on_wait) > max_waits:
                    waits = list(si.on_wait)
                    extra, keep = waits[:-max_waits], waits[-max_waits:]
                    for cs in range(0, len(extra), max_waits):
                        chunk = extra[cs:cs + max_waits]
                        nop = mybir.InstNoOp(name=f"I-wsplit-{k}", ins=[], outs=[])
                        k += 1
                        nop.engine = inst.engine
                        nop.sync_info = bass_rust.SyncInfo(on_wait=chunk, on_update=[])
                        nc.register_instruction(nop)
                        new_list.append(nop)
                    inst.sync_info = bass_rust.SyncInfo(
                        on_wait=keep, on_update=list(si.on_update))
                new_list.append(inst)
            insts[:] = new_list
    return k


def build_bass():
    nc = bass.Bass()

    # ---- per-core DRAM I/O ------------------------------------------------
    rint_d = nc.dram_tensor("rint", [KRIN, 128, BL], BF16, kind="ExternalInput")
    hb_d = nc.dram_tensor("h_b", [BL, H], F32, kind="ExternalInput")
    mem16_d = nc.dram_tensor("mem16", [BL, M, D, S], BF16, kind="ExternalInput")
    mem32_d = nc.dram_tensor("mem32", [BL, M, D, S], F32, kind="ExternalInput")
    wxa_d = nc.dram_tensor("wxa", [3, KXA, 128, H], BF16, kind="ExternalInput")
    wh_d = nc.dram_tensor("wh", [3, KH, 128, H], BF16, kind="ExternalInput")
    wr1_d = nc.dram_tensor("wr1", [KRIN, 128, RH], BF16, kind="ExternalInput")
    wq_d = nc.dram_tensor("wq", [KRIN, 128, D], BF16, kind="ExternalInput")
    wr2_d = nc.dram_tensor("wr2", [RH + 1, M], BF16, kind="ExternalInput")
    wkvg_d = nc.dram_tensor("wkvg", [KKVG, 128, 2 * D + 1], BF16,
                            kind="ExternalInput")
    hout_d = nc.dram_tensor("hout", [BL, H], F32, kind="ExternalOutput")
    memout_d = nc.dram_tensor("memout", [BL, M, D, S], F32, kind="ExternalOutput")

    with tile.TileContext(nc) as tc, \
            tc.tile_pool(name="consts", bufs=1) as consts, \
            tc.tile_pool(name="big", bufs=1) as big, \
            tc.tile_pool(name="wpool", bufs=4) as wpool, \
            tc.tile_pool(name="reps", bufs=2) as reps, \
            tc.tile_pool(name="prods", bufs=2) as prods, \
            tc.tile_pool(name="trees", bufs=1) as trees, \
            tc.tile_pool(name="smalls", bufs=1) as smalls, \
            tc.tile_pool(name="psum", bufs=4, space="PSUM") as psum, \
            tc.tile_pool(name="psumT", bufs=2, space="PSUM") as psumT:

        # ---- constants ----
        ident = consts.tile([128, 128], BF16)
        make_identity(nc, ident)
        const_k9 = consts.tile([128, 128], BF16)  # row0 = 1 (bias row for kvg)
        nc.vector.memset(const_k9, 0.0)
        nc.vector.memset(const_k9[0:1, :], 1.0)
        xat4 = consts.tile([128, 128], BF16)      # ctx.T rows 0-63, row64 = 1
        nc.vector.memset(xat4, 0.0)
        nc.vector.memset(xat4[64:65, :], 1.0)

        # ---- resident inputs ----
        rint = big.tile([128, KRIN, BL], BF16)
        nc.sync.dma_start(out=rint, in_=rint_d[:].rearrange("k p n -> p k n"))
        mem16 = big.tile([BL, M, D, S], BF16)
        nc.sync.dma_start(out=mem16, in_=mem16_d[:])
        h_b = big.tile([BL, H], F32)
        nc.sync.dma_start(out=h_b, in_=hb_d[:])

        # ---- router MLP + q projection -----------------------------------
        with tc.tile_pool(name="projw", bufs=1) as projw:
            wr1 = projw.tile([128, KRIN, RH], BF16)
            nc.sync.dma_start(out=wr1, in_=wr1_d[:].rearrange("k p n -> p k n"))
            wq = projw.tile([128, KRIN, D], BF16)
            nc.sync.dma_start(out=wq, in_=wq_d[:].rearrange("k p n -> p k n"))
            wr2 = projw.tile([RH + 1, M], BF16)
            nc.sync.dma_start(out=wr2, in_=wr2_d[:])

            # r1_pre.T = Wr1.T @ rin  (weights stationary -> transposed out)
            ps_r1t = psumT.tile([RH, BL], F32, tag="pt")
            for k in range(KRIN):
                nc.tensor.matmul(ps_r1t, lhsT=wr1[:, k, :], rhs=rint[:, k, :],
                                 start=(k == 0), stop=(k == KRIN - 1))
            relu1T = smalls.tile([RH + 1, BL], BF16, tag="relu1T")
            nc.vector.memset(relu1T[RH:RH + 1, :], 1.0)  # bias row for Wr2
            nc.scalar.activation(relu1T[0:RH, :], ps_r1t, AF.Relu)

            ps_lg = psumT.tile([BL, M], F32, tag="pt")
            nc.tensor.matmul(ps_lg, lhsT=relu1T, rhs=wr2[:],
                             start=True, stop=True)
            exps4 = smalls.tile([BL, M], F32, tag="exps4")
            nc.scalar.activation(exps4, ps_lg, AF.Exp)
            sum4 = smalls.tile([BL, 1], F32, tag="sum4")
            nc.vector.tensor_reduce(sum4, exps4, axis=AX.X,
                                    op=mybir.AluOpType.add)
            recip4 = smalls.tile([BL, 1], F32, tag="recip4")
            nc.vector.reciprocal(recip4, sum4)
            router_w = smalls.tile([BL, M], F32, tag="router_w")
            nc.vector.tensor_scalar_mul(router_w, exps4, recip4)

            # q = rin @ Wq + bq   -> [BL, D]
            ps_q = psumT.tile([BL, D], F32, tag="pt")
            for k in range(KRIN):
                nc.tensor.matmul(ps_q, lhsT=rint[:, k, :], rhs=wq[:, k, :],
                                 start=(k == 0), stop=(k == KRIN - 1))
            q_bf = smalls.tile([BL, D], BF16, tag="q_bf")
            nc.scalar.activation(q_bf, ps_q, AF.Copy)

        def build_rep(src_bf):
            """[BL, D] -> [BL, D, S] replicated along s via doubling copies."""
            rep = reps.tile([BL, D, S], BF16, tag="rep")
            nc.vector.tensor_copy(rep[:, :, 0], src_bf)
            w = 1
            while w < S:
                nc.vector.tensor_copy(rep[:, :, w:2 * w], rep[:, :, 0:w])
                w *= 2
            return rep

        q_rep = build_rep(q_bf)

        def tree_reduce_d(prod, out_f32_slice):
            """prod [BL, D, S] bf16 -> sum over d -> out[BL, S] (fp32)."""
            a = trees.tile([BL, 32, S], BF16, tag="trA")
            nc.vector.tensor_add(a, prod[:, 0:32, :], prod[:, 32:64, :])
            b = trees.tile([BL, 16, S], BF16, tag="trB")
            nc.vector.tensor_add(b, a[:, 0:16, :], a[:, 16:32, :])
            nc.vector.tensor_add(a[:, 0:8, :], b[:, 0:8, :], b[:, 8:16, :])
            nc.vector.tensor_add(b[:, 0:4, :], a[:, 0:4, :], a[:, 4:8, :])
            nc.vector.tensor_add(a[:, 0:2, :], b[:, 0:2, :], b[:, 2:4, :])
            nc.vector.tensor_add(out_f32_slice, a[:, 0, :], a[:, 1, :])

        def tree_reduce_s(prod, out_f32_slice):
            """prod [BL, D, S] bf16 -> sum over s -> out[BL, D] (fp32)."""
            a = trees.tile([BL, D, 64], BF16, tag="trA")
            nc.vector.tensor_add(a, prod[:, :, 0:64], prod[:, :, 64:128])
            b = trees.tile([BL, D, 32], BF16, tag="trB")
            nc.vector.tensor_add(b, a[:, :, 0:32], a[:, :, 32:64])
            nc.vector.tensor_add(a[:, :, 0:16], b[:, :, 0:16], b[:, :, 16:32])
            nc.vector.tensor_add(b[:, :, 0:8], a[:, :, 0:8], a[:, :, 8:16])
            nc.vector.tensor_add(a[:, :, 0:4], b[:, :, 0:4], b[:, :, 4:8])
            nc.vector.tensor_add(b[:, :, 0:2], a[:, :, 0:2], a[:, :, 2:4])
            nc.vector.tensor_add(out_f32_slice, b[:, :, 0], b[:, :, 1])

        # ---- read attention ----------------------------------------------
        scores = smalls.tile([BL, M, S], F32, tag="scores")
        for m in range(M):
            prod = prods.tile([BL, D, S], BF16, tag="prod")
            nc.vector.tensor_mul(prod, mem16[:, m], q_rep)
            tree_reduce_d(prod, scores[:, m, :])

        exps = smalls.tile([BL, M, S], F32, tag="exps")
        nc.scalar.activation(exps, scores, AF.Exp, scale=1.0 / (D ** 0.5))
        sums = smalls.tile([BL, M], F32, tag="sums")
        nc.vector.tensor_reduce(sums, exps, axis=AX.X, op=mybir.AluOpType.add)
        recip = smalls.tile([BL, M], F32, tag="recip")
        nc.vector.reciprocal(recip, sums)
        rw_recip = smalls.tile([BL, M], F32, tag="rw_recip")
        nc.vector.tensor_mul(rw_recip, router_w, recip)
        alpha = smalls.tile([BL, M, S], BF16, tag="alpha")
        for m in range(M):
            nc.vector.tensor_scalar_mul(alpha[:, m, :], exps[:, m, :],
                                        rw_recip[:, m:m + 1])

        # ctx = sum_{m,s} alpha * mem   (alpha folds router_w and softmax)
        ctx_parts = smalls.tile([BL, M, D], F32, tag="ctx_parts")
        for m in range(M):
            prod = prods.tile([BL, D, S], BF16, tag="prod")
            am = alpha[:, m, :].unsqueeze(1).broadcast_to((BL, D, S))
            nc.vector.tensor_mul(prod, mem16[:, m], am)
            tree_reduce_s(prod, ctx_parts[:, m, :])
        ctx01 = smalls.tile([BL, D], F32, tag="ctx01")
        ctx23 = smalls.tile([BL, D], F32, tag="ctx23")
        ctx = smalls.tile([BL, D], F32, tag="ctx")
        nc.vector.tensor_add(ctx01, ctx_parts[:, 0, :], ctx_parts[:, 1, :])
        nc.vector.tensor_add(ctx23, ctx_parts[:, 2, :], ctx_parts[:, 3, :])
        nc.vector.tensor_add(ctx, ctx01, ctx23)

        # ctx.T into xa tile 4 (rows 0-63), bias row already set
        ctx_bf = smalls.tile([BL, D], BF16, tag="ctx_bf")
        nc.vector.tensor_copy(ctx_bf, ctx)
        ps_ctxT = psumT.tile([D, BL], BF16, tag="ptb")
        nc.tensor.transpose(ps_ctxT, ctx_bf, ident)
        nc.scalar.activation(xat4[0:D, :], ps_ctxT, AF.Copy)

        # ---- GRU -----------------------------------------------------------
        with tc.tile_pool(name="gates", bufs=1) as gates:
            def gate_matmuls(g, rh_T=None):
                """Accumulate xa@Wxa_g + hpart@Wh_g into two psum halves."""
                ph0 = psum.tile([BL, 512], F32, tag="pgate")
                ph1 = psum.tile([BL, 512], F32, tag="pgate")
                nk = KXA + KH
                for k in range(nk):
                    if k < KXA:
                        lhsT = rint[:, k, :] if k < 4 else xat4
                        src = wxa_d[g, k]
                    else:
                        kk = k - KXA
                        lhsT = (rint[:, 4 + kk, :] if rh_T is None
                                else rh_T[:, kk, :])
                        src = wh_d[g, kk]
                    wt = wpool.tile([128, H], BF16, tag="w")
                    nc.sync.dma_start(out=wt, in_=src)
                    for half, ph in ((0, ph0), (1, ph1)):
                        nc.tensor.matmul(
                            ph, lhsT=lhsT,
                            rhs=wt[:, half * 512:(half + 1) * 512],
                            start=(k == 0), stop=(k == nk - 1))
                return ph0, ph1

            ph_r = gate_matmuls(0)
            r32 = gates.tile([BL, H], F32, tag="r32")
            for half in range(2):
                nc.scalar.activation(r32[:, half * 512:(half + 1) * 512],
                                     ph_r[half], AF.Sigmoid)
            ph_z = gate_matmuls(1)
            z32 = gates.tile([BL, H], F32, tag="z32")
            for half in range(2):
                nc.scalar.activation(z32[:, half * 512:(half + 1) * 512],
                                     ph_z[half], AF.Sigmoid)

            rh_bf = gates.tile([BL, H], BF16, tag="rh_bf")
            nc.vector.tensor_mul(rh_bf, r32, h_b)
            rhT = gates.tile([128, KH, BL], BF16, tag="rhT")
            for j in range(KH):
                pt = psumT.tile([128, BL], BF16, tag="ptb")
                nc.tensor.transpose(pt, rh_bf[:, j * 128:(j + 1) * 128], ident)
                nc.scalar.activation(rhT[:, j, :], pt, AF.Copy)

            ph_h = gate_matmuls(2, rh_T=rhT)
            ht32 = gates.tile([BL, H], F32, tag="ht32")
            for half in range(2):
                nc.scalar.activation(ht32[:, half * 512:(half + 1) * 512],
                                     ph_h[half], AF.Tanh)

            # h_new = h + z*(h_tilde - h)
            diff = gates.tile([BL, H], F32, tag="diff")
            nc.vector.tensor_sub(diff, ht32, h_b)
            zd = gates.tile([BL, H], F32, tag="r32")  # reuse r32 slot
            nc.vector.tensor_mul(zd, z32, diff)
            h_new = gates.tile([BL, H], F32, tag="h_new")
            nc.vector.tensor_add(h_new, zd, h_b)
            nc.sync.dma_start(out=hout_d[:], in_=h_new)

            hn_bf = gates.tile([BL, H], BF16, tag="hn_bf")
            nc.vector.tensor_copy(hn_bf, h_new)
            hnT = gates.tile([128, KH, BL], BF16, tag="hnT")
            for j in range(KH):
                pt = psumT.tile([128, BL], BF16, tag="ptb")
                nc.tensor.transpose(pt, hn_bf[:, j * 128:(j + 1) * 128], ident)
                nc.scalar.activation(hnT[:, j, :], pt, AF.Copy)

            wkvg = gates.tile([128, KKVG, 2 * D + 1], BF16, tag="wkvg")
            nc.sync.dma_start(out=wkvg,
                              in_=wkvg_d[:].rearrange("k p n -> p k n"))
            ps_kvg = psumT.tile([BL, 2 * D + 1], F32, tag="pt")
            for k in range(KKVG):
                lhsT = hnT[:, k, :] if k < KH else const_k9
                nc.tensor.matmul(ps_kvg, lhsT=lhsT, rhs=wkvg[:, k, :],
                                 start=(k == 0), stop=(k == KKVG - 1))
            k_bf = smalls.tile([BL, D], BF16, tag="k_bf")
            nc.scalar.activation(k_bf, ps_kvg[:, 0:D], AF.Copy)
            v_bf = smalls.tile([BL, D], BF16, tag="v_bf")
            nc.scalar.activation(v_bf, ps_kvg[:, D:2 * D], AF.Copy)
            gate32 = smalls.tile([BL, 1], F32, tag="gate32")
            nc.scalar.activation(gate32, ps_kvg[:, 2 * D:2 * D + 1], AF.Sigmoid)

        # ---- write attention ----------------------------------------------
        k_rep = build_rep(k_bf)
        wscores = smalls.tile([BL, M, S], F32, tag="scores")
        for m in range(M):
            prod = prods.tile([BL, D, S], BF16, tag="prod")
            nc.vector.tensor_mul(prod, mem16[:, m], k_rep)
            tree_reduce_d(prod, wscores[:, m, :])
        wexps = smalls.tile([BL, M, S], F32, tag="exps")
        nc.scalar.activation(wexps, wscores, AF.Exp, scale=1.0 / (D ** 0.5))
        wsums = smalls.tile([BL, M], F32, tag="sums")
        nc.vector.tensor_reduce(wsums, wexps, axis=AX.X, op=mybir.AluOpType.add)
        wrecip = smalls.tile([BL, M], F32, tag="recip")
        nc.vector.reciprocal(wrecip, wsums)
        rwg = smalls.tile([BL, M], F32, tag="rw_recip")
        nc.vector.tensor_mul(rwg, router_w, wrecip)
        rwg2 = smalls.tile([BL, M], F32, tag="rwg2")
        nc.vector.tensor_scalar_mul(rwg2, rwg, gate32)
        cc = smalls.tile([BL, M, S], BF16, tag="alpha")
        for m in range(M):
            nc.vector.tensor_scalar_mul(cc[:, m, :], wexps[:, m, :],
                                        rwg2[:, m:m + 1])

        # ---- memory update -------------------------------------------------
        v_rep = build_rep(v_bf)
        with tc.tile_pool(name="chunks", bufs=2) as chunks:
            DQ = D // 4  # 16 d-rows per chunk
            for m in range(M):
                outer = prods.tile([BL, D, S], BF16, tag="prod")
                cm = cc[:, m, :].unsqueeze(1).broadcast_to((BL, D, S))
                nc.vector.tensor_mul(outer, cm, v_rep)
                for quarter in range(4):
                    dlo, dhi = quarter * DQ, (quarter + 1) * DQ
                    mi = chunks.tile([BL, DQ, S], F32, tag="min")
                    nc.sync.dma_start(out=mi, in_=mem32_d[:, m, dlo:dhi, :])
                    mo = chunks.tile([BL, DQ, S], F32, tag="mout")
                    eng = nc.vector if quarter < 2 else nc.gpsimd
                    eng.tensor_add(mo, mi, outer[:, dlo:dhi, :])
                    nc.sync.dma_start(out=memout_d[:, m, dlo:dhi, :], in_=mo)

    _split_excess_waits(nc)
    return nc


def _pack_inputs(x, h, memory_state, W_xr, W_hr, b_r, W_xz, W_hz, b_z,
                 W_xh, W_hh, b_h, Wr1, br1, Wr2, br2, Wq, bq, Wk, bk,
                 Wv, bv, Wg, bg):
    f32 = np.float32
    x = np.asarray(x, f32)
    h = np.asarray(h, f32)
    mem = np.asarray(memory_state, f32)

    # rin.T padded to 13 k-tiles with a ones row at 1536 (router/q bias)
    rint = np.zeros((KRIN * 128, B), f32)
    rint[0:IN] = x.T
    rint[IN:IN + H] = h.T
    rint[IN + H] = 1.0
    rint_bf = rint.reshape(KRIN, 128, B).astype(BF_NP)

    # xa weights: [x(512); ctx(64); bias; pad] -> 5 k-tiles
    def pack_wxa(Wf, b):
        out = np.zeros((KXA * 128, H), f32)
        out[0:IN + D] = np.asarray(Wf, f32)
        out[IN + D] = np.asarray(b, f32)
        return out.reshape(KXA, 128, H)
    wxa = np.stack([pack_wxa(W_xr, b_r), pack_wxa(W_xz, b_z),
                    pack_wxa(W_xh, b_h)]).astype(BF_NP)
    wh = np.stack([np.asarray(Wf, f32).reshape(KH, 128, H)
                   for Wf in (W_hr, W_hz, W_hh)]).astype(BF_NP)

    def pack_krin(Wf, b, n):
        out = np.zeros((KRIN * 128, n), f32)
        out[0:IN + H] = np.asarray(Wf, f32)
        out[IN + H] = np.asarray(b, f32)
        return out.reshape(KRIN, 128, n)
    wr1 = pack_krin(Wr1, br1, RH).astype(BF_NP)
    wq = pack_krin(Wq, bq, D).astype(BF_NP)

    wr2 = np.zeros((RH + 1, M), f32)
    wr2[0:RH] = np.asarray(Wr2, f32)
    wr2[RH] = np.asarray(br2, f32)
    wr2 = wr2.astype(BF_NP)

    wkvg = np.zeros((KKVG * 128, 2 * D + 1), f32)
    wkvg[0:H, 0:D] = np.asarray(Wk, f32)
    wkvg[0:H, D:2 * D] = np.asarray(Wv, f32)
    wkvg[0:H, 2 * D] = np.asarray(Wg, f32)[:, 0]
    wkvg[H, 0:D] = np.asarray(bk, f32)
    wkvg[H, D:2 * D] = np.asarray(bv, f32)
    wkvg[H, 2 * D] = np.asarray(bg, f32)[0]
    wkvg = wkvg.reshape(KKVG, 128, 2 * D + 1).astype(BF_NP)

    mem_t = np.ascontiguousarray(mem.transpose(0, 1, 3, 2))  # (B, M, D, S)
    mem16 = mem_t.astype(BF_NP)

    shared = dict(wxa=wxa, wh=wh, wr1=wr1, wq=wq, wr2=wr2, wkvg=wkvg)
    in_maps = []
    for c in range(NCORES):
        sl = slice(c * BL, (c + 1) * BL)
        in_maps.append(dict(
            rint=np.ascontiguousarray(rint_bf[:, :, sl]),
            h_b=np.ascontiguousarray(h[sl]),
            mem16=np.ascontiguousarray(mem16[sl]),
            mem32=np.ascontiguousarray(mem_t[sl]),
            **shared,
        ))
    return in_maps


def kernel(**inputs):
    if "nc" not in _CACHE:
        _CACHE["nc"] = build_bass()
    nc = _CACHE["nc"]
    in_maps = _pack_inputs(**inputs)
    res = run_bass_kernel_spmd(nc, in_maps, core_ids=list(range(NCORES)))
    h_new = np.concatenate([r["hout"] for r in res.results], axis=0)
    mem_t = np.concatenate([r["memout"] for r in res.results], axis=0)
    mem_new = np.ascontiguousarray(mem_t.transpose(0, 1, 3, 2))
    return h_new, mem_new


# revision 12
# speedup vs baseline: 1.6416x; 1.6416x over previous
"""Trainium2 Bass kernel for nn_EnhancedMoMGRUCell.

Data-parallel over batch: 8 NeuronCores x 128 rows each (= SBUF partition
count). Per core, batch rows live on partitions; the per-sample attention
over memory banks runs on DVE/GPSIMD/ACT; dense GRU/router matmuls run on
the PE in bf16 with activations stationary and weights moving (N=512).

Memory-bank tensors use an (m, d, s) on-chip layout so every broadcast
multiply has a unit-stride innermost axis (DVE 2x bf16 mode); reductions
over d and s are log-depth tensor_add trees in bf16 with fp32 final level.

The kernel outputs h_new (fp32) and the rank-1 memory delta
outer[b,m,d,s] = c[b,m,s] * v[b,d] in bf16; the host adds it to the fp32
memory state (exact) and transposes back. GRU gate matmuls over the x/h
k-tiles are emitted before the attention phase consumes DVE so weight
streaming overlaps attention; only the ctx k-tile and the (r*h) tiles
depend on attention results.
"""
import numpy as np
import ml_dtypes

import concourse.bass as bass
import concourse.mybir as mybir
import concourse.tile as tile
import bass_rust
from concourse.bass_utils import run_bass_kernel_spmd
from concourse.masks import make_identity

B, IN, H, M, S, D, RH = 1024, 512, 1024, 4, 128, 64, 32
NCORES = 8
BL = B // NCORES          # 128 batch rows per core
KRIN = 13                 # ceil((IN+H+1)/128): rin + ones row, padded
KXA = 5                   # ceil((IN+D+1)/128): xa + bias row, padded
KH = 8                    # H/128
KKVG = 9                  # H/128 + 1 bias tile
F32 = mybir.dt.float32
BF16 = mybir.dt.bfloat16
AX = mybir.AxisListType
AF = mybir.ActivationFunctionType
BF_NP = ml_dtypes.bfloat16

_CACHE = {}


def _split_excess_waits(nc, max_waits=1):
    """This walrus build rejects instructions with >1 sync-wait; move excess
    waits onto preceding same-engine NoOps (semantically identical)."""
    k = 0
    for f in nc.m.functions:
        for bb in f.blocks:
            insts = bb.instructions
            new_list = []
            for inst in insts:
                si = inst.sync_info
                if si is not None and si.on_wait and len(si.on_wait) > max_waits:
                    waits = list(si.on_wait)
                    extra, keep = waits[:-max_waits], waits[-max_waits:]
                    for cs in range(0, len(extra), max_waits):
                        chunk = extra[cs:cs + max_waits]
                        nop = mybir.InstNoOp(name=f"I-wsplit-{k}", ins=[], outs=[])
                        k += 1
                        nop.engine = inst.engine
                        nop.sync_info = bass_rust.SyncInfo(on_wait=chunk, on_update=[])
                        nc.register_instruction(nop)
                        new_list.append(nop)
                    inst.sync_info = bass_rust.SyncInfo(
                        on_wait=keep, on_update=list(si.on_update))
                new_list.append(inst)
            insts[:] = new_list
    return k


def build_bass():
    nc = bass.Bass()

    # ---- per-core DRAM I/O ------------------------------------------------
    rint_d = nc.dram_tensor("rint", [KRIN, 128, BL], BF16, kind="ExternalInput")
    hb_d = nc.dram_tensor("h_b", [BL, H], F32, kind="ExternalInput")
    mem16_d = nc.dram_tensor("mem16", [BL, M, D, S], BF16, kind="ExternalInput")
    wxa_d = nc.dram_tensor("wxa", [3, KXA, 128, H], BF16, kind="ExternalInput")
    wh_d = nc.dram_tensor("wh", [3, KH, 128, H], BF16, kind="ExternalInput")
    wr1_d = nc.dram_tensor("wr1", [KRIN, 128, RH], BF16, kind="ExternalInput")
    wq_d = nc.dram_tensor("wq", [KRIN, 128, D], BF16, kind="ExternalInput")
    wr2_d = nc.dram_tensor("wr2", [RH + 1, M], BF16, kind="ExternalInput")
    wkvg_d = nc.dram_tensor("wkvg", [KKVG, 128, 2 * D + 1], BF16,
                            kind="ExternalInput")
    hout_d = nc.dram_tensor("hout", [BL, H], F32, kind="ExternalOutput")
    memout_d = nc.dram_tensor("memout", [BL, M, D, S], BF16,
                              kind="ExternalOutput")

    with tile.TileContext(nc) as tc, \
            tc.tile_pool(name="consts", bufs=1) as consts, \
            tc.tile_pool(name="big", bufs=1) as big, \
            tc.tile_pool(name="wpool", bufs=4) as wpool, \
            tc.tile_pool(name="reps", bufs=2) as reps, \
            tc.tile_pool(name="prods", bufs=1) as prods, \
            tc.tile_pool(name="gprods", bufs=1) as gprods, \
            tc.tile_pool(name="trees", bufs=1) as trees, \
            tc.tile_pool(name="gtrees", bufs=1) as gtrees, \
            tc.tile_pool(name="smalls", bufs=1) as smalls, \
            tc.tile_pool(name="gates", bufs=1) as gates, \
            tc.tile_pool(name="psum", bufs=6, space="PSUM") as psum, \
            tc.tile_pool(name="psumT", bufs=2, space="PSUM") as psumT:

        # ---- constants ----
        ident = consts.tile([128, 128], BF16)
        make_identity(nc, ident)
        const_k9 = consts.tile([128, 128], BF16)  # row0 = 1 (bias row for kvg)
        nc.vector.memset(const_k9, 0.0)
        nc.vector.memset(const_k9[0:1, :], 1.0)
        xat4 = consts.tile([128, 128], BF16)      # ctx.T rows 0-63, row64 = 1
        nc.vector.memset(xat4, 0.0)
        nc.vector.memset(xat4[64:65, :], 1.0)

        # ---- resident inputs ----
        rint = big.tile([128, KRIN, BL], BF16)
        nc.sync.dma_start(out=rint, in_=rint_d[:].rearrange("k p n -> p k n"))
        wr1 = big.tile([128, KRIN, RH], BF16)
        nc.sync.dma_start(out=wr1, in_=wr1_d[:].rearrange("k p n -> p k n"))
        wq = big.tile([128, KRIN, D], BF16)
        nc.sync.dma_start(out=wq, in_=wq_d[:].rearrange("k p n -> p k n"))
        wr2 = big.tile([RH + 1, M], BF16)
        nc.sync.dma_start(out=wr2, in_=wr2_d[:])
        h_b = big.tile([BL, H], F32)
        nc.sync.dma_start(out=h_b, in_=hb_d[:])
        wkvg = big.tile([128, KKVG, 2 * D + 1], BF16)
        nc.sync.dma_start(out=wkvg, in_=wkvg_d[:].rearrange("k p n -> p k n"))
        mem16 = big.tile([BL, M, D, S], BF16)
        for m in range(M):
            nc.sync.dma_start(out=mem16[:, m], in_=mem16_d[:, m])

        # ---- router MLP + q projection -----------------------------------
        ps_r1t = psumT.tile([RH, BL], F32, tag="pt")
        for k in range(KRIN):
            nc.tensor.matmul(ps_r1t, lhsT=wr1[:, k, :], rhs=rint[:, k, :],
                             start=(k == 0), stop=(k == KRIN - 1))
        relu1T = smalls.tile([RH + 1, BL], BF16, tag="relu1T")
        nc.vector.memset(relu1T[RH:RH + 1, :], 1.0)  # bias row for Wr2
        nc.scalar.activation(relu1T[0:RH, :], ps_r1t, AF.Relu)

        ps_lg = psumT.tile([BL, M], F32, tag="pt")
        nc.tensor.matmul(ps_lg, lhsT=relu1T, rhs=wr2[:], start=True, stop=True)
        exps4 = smalls.tile([BL, M], F32, tag="exps4")
        nc.scalar.activation(exps4, ps_lg, AF.Exp)
        sum4 = smalls.tile([BL, 1], F32, tag="sum4")
        nc.vector.tensor_reduce(sum4, exps4, axis=AX.X, op=mybir.AluOpType.add)
        recip4 = smalls.tile([BL, 1], F32, tag="recip4")
        nc.vector.reciprocal(recip4, sum4)
        router_w = smalls.tile([BL, M], F32, tag="router_w")
        nc.vector.tensor_scalar_mul(router_w, exps4, recip4)

        ps_q = psumT.tile([BL, D], F32, tag="pt")
        for k in range(KRIN):
            nc.tensor.matmul(ps_q, lhsT=rint[:, k, :], rhs=wq[:, k, :],
                             start=(k == 0), stop=(k == KRIN - 1))
        q_bf = smalls.tile([BL, D], BF16, tag="q_bf")
        nc.scalar.activation(q_bf, ps_q, AF.Copy)

        # ---- GRU gate matmuls over x/h k-tiles (independent of attention;
        # emitted now so weight DMA + PE overlap the attention phase).
        # k-tile order per gate: 4 x-tiles, 8 h-tiles, ctx tile LAST.
        ph = {}
        for g in range(3):
            ph[g] = (psum.tile([BL, 512], F32, tag="pgate", name=f"ph{g}0"),
                     psum.tile([BL, 512], F32, tag="pgate", name=f"ph{g}1"))

        def emit_gate_k(g, lhsT, src_dram, start, stop):
            wt = wpool.tile([128, H], BF16, tag="w")
            nc.sync.dma_start(out=wt, in_=src_dram)
            for half in range(2):
                nc.tensor.matmul(ph[g][half], lhsT=lhsT,
                                 rhs=wt[:, half * 512:(half + 1) * 512],
                                 start=start, stop=stop)

        for g in range(3):      # x-part: k-tiles 0-3 of xa
            for k in range(4):
                emit_gate_k(g, rint[:, k, :], wxa_d[g, k], k == 0, False)
        for g in range(2):      # h-part for r and z (h_tilde's h-part uses r*h)
            for k in range(KH):
                emit_gate_k(g, rint[:, 4 + k, :], wh_d[g, k], False, False)

        # ---- helpers -------------------------------------------------------
        SV, SG = 100, 28   # s-split (DVE/GPSIMD) for score phases
        DV, DG = 50, 14    # d-split (DVE/GPSIMD) for ctx/outer phases
        SCALE = 1.0 / (D ** 0.5)

        def build_rep(src_bf):
            """[BL, D] -> [BL, D, S] replicated along s via doubling copies."""
            rep = reps.tile([BL, D, S], BF16, tag="rep")
            nc.vector.tensor_copy(rep[:, :, 0], src_bf)
            w = 1
            while w < S:
                nc.vector.tensor_copy(rep[:, :, w:2 * w], rep[:, :, 0:w])
                w *= 2
            return rep

        def tree_d(eng, tpool, prod, out_f32, sw, pfx):
            """prod [BL, D, sw] bf16 -> sum over d -> out [BL, sw] fp32."""
            a = tpool.tile([BL, 32, sw], BF16, tag=f"{pfx}A", name=f"{pfx}A_")
            eng.tensor_add(a, prod[:, 0:32, :], prod[:, 32:64, :])
            b = tpool.tile([BL, 16, sw], BF16, tag=f"{pfx}B", name=f"{pfx}B_")
            eng.tensor_add(b, a[:, 0:16, :], a[:, 16:32, :])
            eng.tensor_add(a[:, 0:8, :], b[:, 0:8, :], b[:, 8:16, :])
            eng.tensor_add(b[:, 0:4, :], a[:, 0:4, :], a[:, 4:8, :])
            eng.tensor_add(a[:, 0:2, :], b[:, 0:2, :], b[:, 2:4, :])
            eng.tensor_add(out_f32, a[:, 0, :], a[:, 1, :])

        def tree_s(eng, tpool, prod, out_f32, dw, pfx):
            """prod [BL, dw, S] bf16 -> sum over s -> out [BL, dw] fp32."""
            a = tpool.tile([BL, dw, 64], BF16, tag=f"{pfx}A", name=f"{pfx}A_")
            eng.tensor_add(a, prod[:, :, 0:64], prod[:, :, 64:128])
            b = tpool.tile([BL, dw, 32], BF16, tag=f"{pfx}B", name=f"{pfx}B_")
            eng.tensor_add(b, a[:, :, 0:32], a[:, :, 32:64])
            eng.tensor_add(a[:, :, 0:16], b[:, :, 0:16], b[:, :, 16:32])
            eng.tensor_add(b[:, :, 0:8], a[:, :, 0:8], a[:, :, 8:16])
            eng.tensor_add(a[:, :, 0:4], b[:, :, 0:4], b[:, :, 4:8])
            eng.tensor_add(b[:, :, 0:2], a[:, :, 0:2], a[:, :, 2:4])
            eng.tensor_add(out_f32, b[:, :, 0], b[:, :, 1])

        def scores_unit(m, rep, sc_v, sc_g):
            pv = prods.tile([BL, D, SV], BF16, tag="prod", name="pv")
            nc.vector.tensor_mul(pv[:, :, 0:SV], mem16[:, m, :, 0:SV],
                                 rep[:, :, 0:SV])
            tree_d(nc.vector, trees, pv[:, :, 0:SV], sc_v[:, m, :], SV, "t")
            pg = gprods.tile([BL, D, SG], BF16, tag="gprod", name="pg")
            nc.gpsimd.tensor_mul(pg[:, :, 0:SG], mem16[:, m, :, S - SG:S],
                                 rep[:, :, S - SG:S])
            tree_d(nc.gpsimd, gtrees, pg[:, :, 0:SG], sc_g[:, m, :], SG, "g")

        def softmax_unit(m, sc_v, sc_g, ex_v, ex_g, wvec, alpha_t, sums, rcp, wr):
            """alpha[:, m, :] = wvec[:, m] * softmax over s of scores."""
            nc.scalar.activation(ex_v[:, m, :], sc_v[:, m, :], AF.Exp,
                                 scale=SCALE,
                                 accum_out=sums[:, 2 * m:2 * m + 1])
            nc.scalar.activation(ex_g[:, m, :], sc_g[:, m, :], AF.Exp,
                                 scale=SCALE,
                                 accum_out=sums[:, 2 * m + 1:2 * m + 2])
            nc.vector.tensor_add(sums[:, 2 * m:2 * m + 1],
                                 sums[:, 2 * m:2 * m + 1],
                                 sums[:, 2 * m + 1:2 * m + 2])
            nc.vector.reciprocal(rcp[:, m:m + 1], sums[:, 2 * m:2 * m + 1])
            nc.vector.tensor_mul(wr[:, m:m + 1], wvec[:, m:m + 1],
                                 rcp[:, m:m + 1])
            nc.scalar.activation(alpha_t[:, m, 0:SV], ex_v[:, m, :], AF.Copy,
                                 scale=wr[:, m:m + 1])
            nc.scalar.activation(alpha_t[:, m, SV:S], ex_g[:, m, :], AF.Copy,
                                 scale=wr[:, m:m + 1])

        def weighted_sum_unit(m, alpha_t, out_parts):
            av = alpha_t[:, m, :].unsqueeze(1).broadcast_to((BL, DV, S))
            pv = prods.tile([BL, DV, S], BF16, tag="prod2", name="pv2")
            nc.vector.tensor_mul(pv, mem16[:, m, 0:DV, :], av)
            tree_s(nc.vector, trees, pv, out_parts[:, m, 0:DV], DV, "t")
            ag = alpha_t[:, m, :].unsqueeze(1).broadcast_to((BL, DG, S))
            pg = gprods.tile([BL, DG, S], BF16, tag="gprod2", name="pg2")
            nc.gpsimd.tensor_mul(pg, mem16[:, m, DV:D, :], ag)
            tree_s(nc.gpsimd, gtrees, pg, out_parts[:, m, DV:D], DG, "g")

        # ---- read attention (per-m fused score+softmax+ctx) ---------------
        q_rep = build_rep(q_bf)
        scores_v = smalls.tile([BL, M, SV], F32, tag="scores_v")
        scores_g = smalls.tile([BL, M, SG], F32, tag="scores_g")
        exps_v = smalls.tile([BL, M, SV], F32, tag="exps_v")
        exps_g = smalls.tile([BL, M, SG], F32, tag="exps_g")
        sums = smalls.tile([BL, 2 * M], F32, tag="sums")
        recip = smalls.tile([BL, M], F32, tag="recip")
        rw_recip = smalls.tile([BL, M], F32, tag="rw_recip")
        alpha = smalls.tile([BL, M, S], BF16, tag="alpha")
        ctx_parts = smalls.tile([BL, M, D], F32, tag="ctx_parts")
        for m in range(M):
            scores_unit(m, q_rep, scores_v, scores_g)
        for m in range(M):
            softmax_unit(m, scores_v, scores_g, exps_v, exps_g, router_w,
                         alpha, sums, recip, rw_recip)
            weighted_sum_unit(m, alpha, ctx_parts)

        ctx01 = smalls.tile([BL, D], F32, tag="ctx01")
        ctx23 = smalls.tile([BL, D], F32, tag="ctx23")
        ctx = smalls.tile([BL, D], F32, tag="ctx")
        nc.vector.tensor_add(ctx01, ctx_parts[:, 0, :], ctx_parts[:, 1, :])
        nc.vector.tensor_add(ctx23, ctx_parts[:, 2, :], ctx_parts[:, 3, :])
        nc.vector.tensor_add(ctx, ctx01, ctx23)

        ctx_bf = smalls.tile([BL, D], BF16, tag="ctx_bf")
        nc.vector.tensor_copy(ctx_bf, ctx)
        ps_ctxT = psumT.tile([D, BL], BF16, tag="pt")
        nc.tensor.transpose(ps_ctxT, ctx_bf, ident)
        nc.scalar.activation(xat4[0:D, :], ps_ctxT, AF.Copy)

        # ---- finish GRU ----------------------------------------------------
        emit_gate_k(0, xat4, wxa_d[0, 4], False, True)
        emit_gate_k(1, xat4, wxa_d[1, 4], False, True)

        r32 = gates.tile([BL, H], F32, tag="r32")
        for half in range(2):
            nc.scalar.activation(r32[:, half * 512:(half + 1) * 512],
                                 ph[0][half], AF.Sigmoid)
        z32 = gates.tile([BL, H], F32, tag="z32")
        for half in range(2):
            nc.scalar.activation(z32[:, half * 512:(half + 1) * 512],
                                 ph[1][half], AF.Sigmoid)

        rh_bf = gates.tile([BL, H], BF16, tag="rh_bf")
        nc.vector.tensor_mul(rh_bf, r32, h_b)
        rhT = gates.tile([128, KH, BL], BF16, tag="rhT")
        for j in range(KH):
            pt = psumT.tile([128, BL], BF16, tag="pt", name=f"ptr{j}")
            nc.tensor.transpose(pt, rh_bf[:, j * 128:(j + 1) * 128], ident)
            nc.scalar.activation(rhT[:, j, :], pt, AF.Copy)

        emit_gate_k(2, xat4, wxa_d[2, 4], False, False)
        for k in range(KH):
            emit_gate_k(2, rhT[:, k, :], wh_d[2, k], False, k == KH - 1)
        ht32 = gates.tile([BL, H], F32, tag="ht32")
        for half in range(2):
            nc.scalar.activation(ht32[:, half * 512:(half + 1) * 512],
                                 ph[2][half], AF.Tanh)

        # h_new = h + z*(h_tilde - h)
        diff = gates.tile([BL, H], F32, tag="diff")
        nc.vector.tensor_sub(diff, ht32, h_b)
        zd = gates.tile([BL, H], F32, tag="r32")  # reuse r32 slot
        nc.vector.tensor_mul(zd, z32, diff)
        h_new = gates.tile([BL, H], F32, tag="h_new")
        nc.vector.tensor_add(h_new, zd, h_b)
        nc.sync.dma_start(out=hout_d[:], in_=h_new)

        hn_bf = gates.tile([BL, H], BF16, tag="rh_bf")
        nc.vector.tensor_copy(hn_bf, h_new)
        hnT = gates.tile([128, KH, BL], BF16, tag="rhT")
        for j in range(KH):
            pt = psumT.tile([128, BL], BF16, tag="pt", name=f"pth{j}")
            nc.tensor.transpose(pt, hn_bf[:, j * 128:(j + 1) * 128], ident)
            nc.scalar.activation(hnT[:, j, :], pt, AF.Copy)

        ps_kvg = psumT.tile([BL, 2 * D + 1], F32, tag="pt")
        for k in range(KKVG):
            lhsT = hnT[:, k, :] if k < KH else const_k9
            nc.tensor.matmul(ps_kvg, lhsT=lhsT, rhs=wkvg[:, k, :],
                             start=(k == 0), stop=(k == KKVG - 1))
        k_bf = smalls.tile([BL, D], BF16, tag="k_bf")
        nc.scalar.activation(k_bf, ps_kvg[:, 0:D], AF.Copy)
        v_bf = smalls.tile([BL, D], BF16, tag="v_bf")
        nc.scalar.activation(v_bf, ps_kvg[:, D:2 * D], AF.Copy)
        gate32 = smalls.tile([BL, 1], F32, tag="gate32")
        nc.scalar.activation(gate32, ps_kvg[:, 2 * D:2 * D + 1], AF.Sigmoid)

        # ---- write attention + memory delta (per-m fused) ------------------
        k_rep = build_rep(k_bf)
        v_rep = build_rep(v_bf)
        rwgate = smalls.tile([BL, M], F32, tag="rwgate")
        nc.vector.tensor_scalar_mul(rwgate, router_w, gate32)
        ccs = smalls.tile([BL, M, S], BF16, tag="alpha")
        for m in range(M):
            scores_unit(m, k_rep, scores_v, scores_g)
        for m in range(M):
            softmax_unit(m, scores_v, scores_g, exps_v, exps_g, rwgate,
                         ccs, sums, recip, rw_recip)
            cm_v = ccs[:, m, :].unsqueeze(1).broadcast_to((BL, DV, S))
            outer_v = prods.tile([BL, DV, S], BF16, tag="prod2", name="ov")
            nc.vector.tensor_mul(outer_v, cm_v, v_rep[:, 0:DV, :])
            nc.sync.dma_start(out=memout_d[:, m, 0:DV, :], in_=outer_v)
            cm_g = ccs[:, m, :].unsqueeze(1).broadcast_to((BL, DG, S))
            outer_g = gprods.tile([BL, DG, S], BF16, tag="gprod2", name="og")
            nc.gpsimd.tensor_mul(outer_g, cm_g, v_rep[:, DV:D, :])
            nc.sync.dma_start(out=memout_d[:, m, DV:D, :], in_=outer_g)

    _split_excess_waits(nc)
    return nc


def _pack_inputs(x, h, memory_state, W_xr, W_hr, b_r, W_xz, W_hz, b_z,
                 W_xh, W_hh, b_h, Wr1, br1, Wr2, br2, Wq, bq, Wk, bk,
                 Wv, bv, Wg, bg):
    f32 = np.float32
    x = np.asarray(x, f32)
    h = np.asarray(h, f32)
    mem = np.asarray(memory_state, f32)

    # rin.T padded to 13 k-tiles with a ones row at 1536 (router/q bias)
    rint = np.zeros((KRIN * 128, B), f32)
    rint[0:IN] = x.T
    rint[IN:IN + H] = h.T
    rint[IN + H] = 1.0
    rint_bf = rint.reshape(KRIN, 128, B).astype(BF_NP)

    # xa weights: [x(512); ctx(64); bias; pad] -> 5 k-tiles
    def pack_wxa(Wf, b):
        out = np.zeros((KXA * 128, H), f32)
        out[0:IN + D] = np.asarray(Wf, f32)
        out[IN + D] = np.asarray(b, f32)
        return out.reshape(KXA, 128, H)
    wxa = np.stack([pack_wxa(W_xr, b_r), pack_wxa(W_xz, b_z),
                    pack_wxa(W_xh, b_h)]).astype(BF_NP)
    wh = np.stack([np.asarray(Wf, f32).reshape(KH, 128, H)
                   for Wf in (W_hr, W_hz, W_hh)]).astype(BF_NP)

    def pack_krin(Wf, b, n):
        out = np.zeros((KRIN * 128, n), f32)
        out[0:IN + H] = np.asarray(Wf, f32)
        out[IN + H] = np.asarray(b, f32)
        return out.reshape(KRIN, 128, n)
    wr1 = pack_krin(Wr1, br1, RH).astype(BF_NP)
    wq = pack_krin(Wq, bq, D).astype(BF_NP)

    wr2 = np.zeros((RH + 1, M), f32)
    wr2[0:RH] = np.asarray(Wr2, f32)
    wr2[RH] = np.asarray(br2, f32)
    wr2 = wr2.astype(BF_NP)

    wkvg = np.zeros((KKVG * 128, 2 * D + 1), f32)
    wkvg[0:H, 0:D] = np.asarray(Wk, f32)
    wkvg[0:H, D:2 * D] = np.asarray(Wv, f32)
    wkvg[0:H, 2 * D] = np.asarray(Wg, f32)[:, 0]
    wkvg[H, 0:D] = np.asarray(bk, f32)
    wkvg[H, D:2 * D] = np.asarray(bv, f32)
    wkvg[H, 2 * D] = np.asarray(bg, f32)[0]
    wkvg = wkvg.reshape(KKVG, 128, 2 * D + 1).astype(BF_NP)

    mem_t = np.ascontiguousarray(mem.transpose(0, 1, 3, 2))  # (B, M, D, S)
    mem16 = mem_t.astype(BF_NP)

    shared = dict(wxa=wxa, wh=wh, wr1=wr1, wq=wq, wr2=wr2, wkvg=wkvg)
    in_maps = []
    for c in range(NCORES):
        sl = slice(c * BL, (c + 1) * BL)
        in_maps.append(dict(
            rint=np.ascontiguousarray(rint_bf[:, :, sl]),
            h_b=np.ascontiguousarray(h[sl]),
            mem16=np.ascontiguousarray(mem16[sl]),
            **shared,
        ))
    return in_maps


def kernel(**inputs):
    if "nc" not in _CACHE:
        _CACHE["nc"] = build_bass()
    nc = _CACHE["nc"]
    in_maps = _pack_inputs(**inputs)
    res = run_bass_kernel_spmd(nc, in_maps, core_ids=list(range(NCORES)))
    h_new = np.concatenate([r["hout"] for r in res.results], axis=0)
    outer_t = np.concatenate([np.asarray(r["memout"], np.float32)
                              for r in res.results], axis=0)  # (B, M, D, S)
    mem_new = np.asarray(inputs["memory_state"], np.float32) \
        + outer_t.transpose(0, 1, 3, 2)
    return h_new, mem_new


# revision 13
# speedup vs baseline: 1.7517x; 1.0671x over previous
"""Trainium2 Bass kernel for nn_EnhancedMoMGRUCell.

Data-parallel over batch: 8 NeuronCores x 128 rows each (= SBUF partition
count). Per core, batch rows live on partitions; the per-sample attention
over memory banks runs on DVE/GPSIMD/ACT; dense GRU/router matmuls run on
the PE in bf16 with activations stationary and weights moving (N=512).

Memory-bank tensors use an (m, d, s) on-chip layout so every broadcast
multiply has a unit-stride innermost axis (DVE 2x bf16 mode); reductions
over d and s are log-depth tensor_add trees in bf16 with fp32 final level.

The kernel outputs h_new (fp32) and the rank-1 memory delta
outer[b,m,d,s] = c[b,m,s] * v[b,d] in bf16; the host adds it to the fp32
memory state (exact) and transposes back. GRU gate matmuls over the x/h
k-tiles are emitted before the attention phase consumes DVE so weight
streaming overlaps attention; only the ctx k-tile and the (r*h) tiles
depend on attention results.
"""
import numpy as np
import ml_dtypes

import concourse.bass as bass
import concourse.mybir as mybir
import concourse.tile as tile
import bass_rust
from concourse.bass_utils import run_bass_kernel_spmd
from concourse.masks import make_identity

B, IN, H, M, S, D, RH = 1024, 512, 1024, 4, 128, 64, 32
NCORES = 8
BL = B // NCORES          # 128 batch rows per core
KRIN = 13                 # ceil((IN+H+1)/128): rin + ones row, padded
KXA = 5                   # ceil((IN+D+1)/128): xa + bias row, padded
KH = 8                    # H/128
KKVG = 9                  # H/128 + 1 bias tile
F32 = mybir.dt.float32
BF16 = mybir.dt.bfloat16
AX = mybir.AxisListType
AF = mybir.ActivationFunctionType
BF_NP = ml_dtypes.bfloat16

_CACHE = {}


def _split_excess_waits(nc, max_waits=1):
    """This walrus build rejects instructions with >1 sync-wait; move excess
    waits onto preceding same-engine NoOps (semantically identical)."""
    k = 0
    for f in nc.m.functions:
        for bb in f.blocks:
            insts = bb.instructions
            new_list = []
            for inst in insts:
                si = inst.sync_info
                if si is not None and si.on_wait and len(si.on_wait) > max_waits:
                    waits = list(si.on_wait)
                    extra, keep = waits[:-max_waits], waits[-max_waits:]
                    for cs in range(0, len(extra), max_waits):
                        chunk = extra[cs:cs + max_waits]
                        nop = mybir.InstNoOp(name=f"I-wsplit-{k}", ins=[], outs=[])
                        k += 1
                        nop.engine = inst.engine
                        nop.sync_info = bass_rust.SyncInfo(on_wait=chunk, on_update=[])
                        nc.register_instruction(nop)
                        new_list.append(nop)
                    inst.sync_info = bass_rust.SyncInfo(
                        on_wait=keep, on_update=list(si.on_update))
                new_list.append(inst)
            insts[:] = new_list
    return k


def build_bass():
    nc = bass.Bass()

    # ---- per-core DRAM I/O ------------------------------------------------
    rint_d = nc.dram_tensor("rint", [KRIN, 128, BL], BF16, kind="ExternalInput")
    hb_d = nc.dram_tensor("h_b", [BL, H], F32, kind="ExternalInput")
    mem16_d = nc.dram_tensor("mem16", [BL, M, D, S], BF16, kind="ExternalInput")
    wxa_d = nc.dram_tensor("wxa", [3, KXA, 128, H], BF16, kind="ExternalInput")
    wh_d = nc.dram_tensor("wh", [3, KH, 128, H], BF16, kind="ExternalInput")
    wr1_d = nc.dram_tensor("wr1", [KRIN, 128, RH], BF16, kind="ExternalInput")
    wq_d = nc.dram_tensor("wq", [KRIN, 128, D], BF16, kind="ExternalInput")
    wr2_d = nc.dram_tensor("wr2", [RH + 1, M], BF16, kind="ExternalInput")
    wkvg_d = nc.dram_tensor("wkvg", [KKVG, 128, 2 * D + 1], BF16,
                            kind="ExternalInput")
    hout_d = nc.dram_tensor("hout", [BL, H], F32, kind="ExternalOutput")
    memout_d = nc.dram_tensor("memout", [BL, M, D, S], BF16,
                              kind="ExternalOutput")

    with tile.TileContext(nc) as tc, \
            tc.tile_pool(name="consts", bufs=1) as consts, \
            tc.tile_pool(name="big", bufs=1) as big, \
            tc.tile_pool(name="wpool", bufs=4) as wpool, \
            tc.tile_pool(name="reps", bufs=2) as reps, \
            tc.tile_pool(name="prods", bufs=1) as prods, \
            tc.tile_pool(name="gprods", bufs=1) as gprods, \
            tc.tile_pool(name="trees", bufs=1) as trees, \
            tc.tile_pool(name="gtrees", bufs=1) as gtrees, \
            tc.tile_pool(name="smalls", bufs=1) as smalls, \
            tc.tile_pool(name="gates", bufs=1) as gates, \
            tc.tile_pool(name="psum", bufs=6, space="PSUM") as psum, \
            tc.tile_pool(name="psumT", bufs=2, space="PSUM") as psumT:

        # ---- constants ----
        ident = consts.tile([128, 128], BF16)
        make_identity(nc, ident)
        const_k9 = consts.tile([128, 128], BF16)  # row0 = 1 (bias row for kvg)
        nc.vector.memset(const_k9, 0.0)
        nc.vector.memset(const_k9[0:1, :], 1.0)
        xat4 = consts.tile([128, 128], BF16)      # ctx.T rows 0-63, row64 = 1
        nc.vector.memset(xat4, 0.0)
        nc.vector.memset(xat4[64:65, :], 1.0)

        # ---- resident inputs ----
        rint = big.tile([128, KRIN, BL], BF16)
        nc.sync.dma_start(out=rint, in_=rint_d[:].rearrange("k p n -> p k n"))
        wr1 = big.tile([128, KRIN, RH], BF16)
        nc.sync.dma_start(out=wr1, in_=wr1_d[:].rearrange("k p n -> p k n"))
        wq = big.tile([128, KRIN, D], BF16)
        nc.sync.dma_start(out=wq, in_=wq_d[:].rearrange("k p n -> p k n"))
        wr2 = big.tile([RH + 1, M], BF16)
        nc.sync.dma_start(out=wr2, in_=wr2_d[:])
        h_b = big.tile([BL, H], F32)
        nc.sync.dma_start(out=h_b, in_=hb_d[:])
        wkvg = big.tile([128, KKVG, 2 * D + 1], BF16)
        nc.sync.dma_start(out=wkvg, in_=wkvg_d[:].rearrange("k p n -> p k n"))
        mem16 = big.tile([BL, M, D, S], BF16)
        for m in range(M):
            nc.sync.dma_start(out=mem16[:, m], in_=mem16_d[:, m])

        # ---- router MLP + q projection -----------------------------------
        ps_r1t = psumT.tile([RH, BL], F32, tag="pt")
        for k in range(KRIN):
            nc.tensor.matmul(ps_r1t, lhsT=wr1[:, k, :], rhs=rint[:, k, :],
                             start=(k == 0), stop=(k == KRIN - 1))
        relu1T = smalls.tile([RH + 1, BL], BF16, tag="relu1T")
        nc.vector.memset(relu1T[RH:RH + 1, :], 1.0)  # bias row for Wr2
        nc.scalar.activation(relu1T[0:RH, :], ps_r1t, AF.Relu)

        ps_lg = psumT.tile([BL, M], F32, tag="pt")
        nc.tensor.matmul(ps_lg, lhsT=relu1T, rhs=wr2[:], start=True, stop=True)
        exps4 = smalls.tile([BL, M], F32, tag="exps4")
        nc.scalar.activation(exps4, ps_lg, AF.Exp)
        sum4 = smalls.tile([BL, 1], F32, tag="sum4")
        nc.vector.tensor_reduce(sum4, exps4, axis=AX.X, op=mybir.AluOpType.add)
        recip4 = smalls.tile([BL, 1], F32, tag="recip4")
        nc.vector.reciprocal(recip4, sum4)
        router_w = smalls.tile([BL, M], F32, tag="router_w")
        nc.vector.tensor_scalar_mul(router_w, exps4, recip4)

        ps_q = psumT.tile([BL, D], F32, tag="pt")
        for k in range(KRIN):
            nc.tensor.matmul(ps_q, lhsT=rint[:, k, :], rhs=wq[:, k, :],
                             start=(k == 0), stop=(k == KRIN - 1))
        q_bf = smalls.tile([BL, D], BF16, tag="q_bf")
        nc.scalar.activation(q_bf, ps_q, AF.Copy)

        # ---- GRU gate matmuls over x/h k-tiles (independent of attention;
        # emitted now so weight DMA + PE overlap the attention phase).
        # k-tile order per gate: 4 x-tiles, 8 h-tiles, ctx tile LAST.
        ph = {}
        for g in range(3):
            ph[g] = (psum.tile([BL, 512], F32, tag="pgate", name=f"ph{g}0"),
                     psum.tile([BL, 512], F32, tag="pgate", name=f"ph{g}1"))

        def emit_gate_k(g, lhsT, src_dram, start, stop):
            wt = wpool.tile([128, H], BF16, tag="w")
            nc.sync.dma_start(out=wt, in_=src_dram)
            for half in range(2):
                nc.tensor.matmul(ph[g][half], lhsT=lhsT,
                                 rhs=wt[:, half * 512:(half + 1) * 512],
                                 start=start, stop=stop)

        for g in range(3):      # x-part: k-tiles 0-3 of xa
            for k in range(4):
                emit_gate_k(g, rint[:, k, :], wxa_d[g, k], k == 0, False)
        for g in range(2):      # h-part for r and z (h_tilde's h-part uses r*h)
            for k in range(KH):
                emit_gate_k(g, rint[:, 4 + k, :], wh_d[g, k], False, False)

        # ---- helpers -------------------------------------------------------
        SV, SG = 100, 28   # s-split (DVE/GPSIMD) for score phases
        DV, DG = 50, 14    # d-split (DVE/GPSIMD) for ctx/outer phases
        SCALE = 1.0 / (D ** 0.5)

        def build_rep(src_bf):
            """[BL, D] -> [BL, D, S] replicated along s via doubling copies."""
            rep = reps.tile([BL, D, S], BF16, tag="rep")
            nc.vector.tensor_copy(rep[:, :, 0], src_bf)
            w = 1
            while w < S:
                nc.vector.tensor_copy(rep[:, :, w:2 * w], rep[:, :, 0:w])
                w *= 2
            return rep

        def tree_d(eng, tpool, prod, out_f32, sw, pfx):
            """prod [BL, D, sw] bf16 -> sum over d -> out [BL, sw] fp32."""
            a = tpool.tile([BL, 32, sw], BF16, tag=f"{pfx}A", name=f"{pfx}A_")
            eng.tensor_add(a, prod[:, 0:32, :], prod[:, 32:64, :])
            b = tpool.tile([BL, 16, sw], BF16, tag=f"{pfx}B", name=f"{pfx}B_")
            eng.tensor_add(b, a[:, 0:16, :], a[:, 16:32, :])
            eng.tensor_add(a[:, 0:8, :], b[:, 0:8, :], b[:, 8:16, :])
            eng.tensor_add(b[:, 0:4, :], a[:, 0:4, :], a[:, 4:8, :])
            eng.tensor_add(a[:, 0:2, :], b[:, 0:2, :], b[:, 2:4, :])
            eng.tensor_add(out_f32, a[:, 0, :], a[:, 1, :])

        def tree_s(eng, tpool, prod, out_f32, dw, pfx):
            """prod [BL, dw, S] bf16 -> sum over s -> out [BL, dw] fp32."""
            a = tpool.tile([BL, dw, 64], BF16, tag=f"{pfx}A", name=f"{pfx}A_")
            eng.tensor_add(a, prod[:, :, 0:64], prod[:, :, 64:128])
            b = tpool.tile([BL, dw, 32], BF16, tag=f"{pfx}B", name=f"{pfx}B_")
            eng.tensor_add(b, a[:, :, 0:32], a[:, :, 32:64])
            eng.tensor_add(a[:, :, 0:16], b[:, :, 0:16], b[:, :, 16:32])
            eng.tensor_add(b[:, :, 0:8], a[:, :, 0:8], a[:, :, 8:16])
            eng.tensor_add(a[:, :, 0:4], b[:, :, 0:4], b[:, :, 4:8])
            eng.tensor_add(b[:, :, 0:2], a[:, :, 0:2], a[:, :, 2:4])
            eng.tensor_add(out_f32, b[:, :, 0], b[:, :, 1])

        def scores_unit(m, rep, sc_v, sc_g):
            pv = prods.tile([BL, D, SV], BF16, tag="prod", name="pv")
            nc.vector.tensor_mul(pv[:, :, 0:SV], mem16[:, m, :, 0:SV],
                                 rep[:, :, 0:SV])
            tree_d(nc.vector, trees, pv[:, :, 0:SV], sc_v[:, m, :], SV, "t")
            pg = gprods.tile([BL, D, SG], BF16, tag="gprod", name="pg")
            nc.gpsimd.tensor_mul(pg[:, :, 0:SG], mem16[:, m, :, S - SG:S],
                                 rep[:, :, S - SG:S])
            tree_d(nc.gpsimd, gtrees, pg[:, :, 0:SG], sc_g[:, m, :], SG, "g")

        def softmax_unit(m, sc_v, sc_g, ex_v, ex_g, wvec, alpha_t, sums, rcp, wr):
            """alpha[:, m, :] = wvec[:, m] * softmax over s of scores."""
            nc.scalar.activation(ex_v[:, m, :], sc_v[:, m, :], AF.Exp,
                                 scale=SCALE,
                                 accum_out=sums[:, 2 * m:2 * m + 1])
            nc.scalar.activation(ex_g[:, m, :], sc_g[:, m, :], AF.Exp,
                                 scale=SCALE,
                                 accum_out=sums[:, 2 * m + 1:2 * m + 2])
            nc.vector.tensor_add(sums[:, 2 * m:2 * m + 1],
                                 sums[:, 2 * m:2 * m + 1],
                                 sums[:, 2 * m + 1:2 * m + 2])
            nc.vector.reciprocal(rcp[:, m:m + 1], sums[:, 2 * m:2 * m + 1])
            nc.vector.tensor_mul(wr[:, m:m + 1], wvec[:, m:m + 1],
                                 rcp[:, m:m + 1])
            nc.scalar.activation(alpha_t[:, m, 0:SV], ex_v[:, m, :], AF.Copy,
                                 scale=wr[:, m:m + 1])
            nc.scalar.activation(alpha_t[:, m, SV:S], ex_g[:, m, :], AF.Copy,
                                 scale=wr[:, m:m + 1])

        def weighted_sum_unit(m, alpha_t, out_parts):
            av = alpha_t[:, m, :].unsqueeze(1).broadcast_to((BL, DV, S))
            pv = prods.tile([BL, DV, S], BF16, tag="prod2", name="pv2", bufs=2)
            nc.vector.tensor_mul(pv, mem16[:, m, 0:DV, :], av)
            tree_s(nc.vector, trees, pv, out_parts[:, m, 0:DV], DV, "t")
            ag = alpha_t[:, m, :].unsqueeze(1).broadcast_to((BL, DG, S))
            pg = gprods.tile([BL, DG, S], BF16, tag="gprod2", name="pg2", bufs=2)
            nc.gpsimd.tensor_mul(pg, mem16[:, m, DV:D, :], ag)
            tree_s(nc.gpsimd, gtrees, pg, out_parts[:, m, DV:D], DG, "g")

        # ---- read attention (per-m fused score+softmax+ctx) ---------------
        q_rep = build_rep(q_bf)
        scores_v = smalls.tile([BL, M, SV], F32, tag="scores_v")
        scores_g = smalls.tile([BL, M, SG], F32, tag="scores_g")
        exps_v = smalls.tile([BL, M, SV], F32, tag="exps_v")
        exps_g = smalls.tile([BL, M, SG], F32, tag="exps_g")
        sums = smalls.tile([BL, 2 * M], F32, tag="sums")
        recip = smalls.tile([BL, M], F32, tag="recip")
        rw_recip = smalls.tile([BL, M], F32, tag="rw_recip")
        alpha = smalls.tile([BL, M, S], BF16, tag="alpha")
        ctx_parts = smalls.tile([BL, M, D], F32, tag="ctx_parts")
        for m in range(M):
            scores_unit(m, q_rep, scores_v, scores_g)
        for m in range(M):
            softmax_unit(m, scores_v, scores_g, exps_v, exps_g, router_w,
                         alpha, sums, recip, rw_recip)
            weighted_sum_unit(m, alpha, ctx_parts)

        ctx01 = smalls.tile([BL, D], F32, tag="ctx01")
        ctx23 = smalls.tile([BL, D], F32, tag="ctx23")
        nc.vector.tensor_add(ctx01, ctx_parts[:, 0, :], ctx_parts[:, 1, :])
        nc.vector.tensor_add(ctx23, ctx_parts[:, 2, :], ctx_parts[:, 3, :])
        ctx_bf = smalls.tile([BL, D], BF16, tag="ctx_bf")
        nc.vector.tensor_add(ctx_bf, ctx01, ctx23)
        ps_ctxT = psumT.tile([D, BL], BF16, tag="pt")
        nc.tensor.transpose(ps_ctxT, ctx_bf, ident)
        nc.scalar.activation(xat4[0:D, :], ps_ctxT, AF.Copy)

        # ---- finish GRU ----------------------------------------------------
        emit_gate_k(0, xat4, wxa_d[0, 4], False, True)
        emit_gate_k(1, xat4, wxa_d[1, 4], False, True)

        r32 = gates.tile([BL, H], F32, tag="r32")
        for half in range(2):
            nc.scalar.activation(r32[:, half * 512:(half + 1) * 512],
                                 ph[0][half], AF.Sigmoid)
        z32 = gates.tile([BL, H], F32, tag="z32")
        for half in range(2):
            nc.scalar.activation(z32[:, half * 512:(half + 1) * 512],
                                 ph[1][half], AF.Sigmoid)

        rh_bf = gates.tile([BL, H], BF16, tag="rh_bf")
        nc.vector.tensor_mul(rh_bf, r32, h_b)
        rhT = gates.tile([128, KH, BL], BF16, tag="rhT")
        for j in range(KH):
            pt = psumT.tile([128, BL], BF16, tag="pt", name=f"ptr{j}")
            nc.tensor.transpose(pt, rh_bf[:, j * 128:(j + 1) * 128], ident)
            nc.scalar.activation(rhT[:, j, :], pt, AF.Copy)

        emit_gate_k(2, xat4, wxa_d[2, 4], False, False)
        for k in range(KH):
            emit_gate_k(2, rhT[:, k, :], wh_d[2, k], False, k == KH - 1)
        ht32 = gates.tile([BL, H], F32, tag="ht32")
        for half in range(2):
            nc.scalar.activation(ht32[:, half * 512:(half + 1) * 512],
                                 ph[2][half], AF.Tanh)

        # h_new = h + z*(h_tilde - h)
        diff = gates.tile([BL, H], F32, tag="diff")
        nc.vector.tensor_sub(diff, ht32, h_b)
        zd = gates.tile([BL, H], F32, tag="r32")  # reuse r32 slot
        nc.vector.tensor_mul(zd, z32, diff)
        h_new = gates.tile([BL, H], F32, tag="diff")
        nc.vector.tensor_add(h_new, zd, h_b)
        nc.sync.dma_start(out=hout_d[:], in_=h_new)

        hn_bf = gates.tile([BL, H], BF16, tag="rh_bf")
        nc.vector.tensor_copy(hn_bf, h_new)
        hnT = gates.tile([128, KH, BL], BF16, tag="rhT")
        for j in range(KH):
            pt = psumT.tile([128, BL], BF16, tag="pt", name=f"pth{j}")
            nc.tensor.transpose(pt, hn_bf[:, j * 128:(j + 1) * 128], ident)
            nc.scalar.activation(hnT[:, j, :], pt, AF.Copy)

        ps_kvg = psumT.tile([BL, 2 * D + 1], F32, tag="pt")
        for k in range(KKVG):
            lhsT = hnT[:, k, :] if k < KH else const_k9
            nc.tensor.matmul(ps_kvg, lhsT=lhsT, rhs=wkvg[:, k, :],
                             start=(k == 0), stop=(k == KKVG - 1))
        k_bf = smalls.tile([BL, D], BF16, tag="k_bf")
        nc.scalar.activation(k_bf, ps_kvg[:, 0:D], AF.Copy)
        v_bf = smalls.tile([BL, D], BF16, tag="v_bf")
        nc.scalar.activation(v_bf, ps_kvg[:, D:2 * D], AF.Copy)
        gate32 = smalls.tile([BL, 1], F32, tag="gate32")
        nc.scalar.activation(gate32, ps_kvg[:, 2 * D:2 * D + 1], AF.Sigmoid)

        # ---- write attention + memory delta (per-m fused) ------------------
        k_rep = build_rep(k_bf)
        rwgate = smalls.tile([BL, M], F32, tag="rwgate")
        nc.vector.tensor_scalar_mul(rwgate, router_w, gate32)
        ccs = smalls.tile([BL, M, S], BF16, tag="alpha")
        for m in range(M):
            scores_unit(m, k_rep, scores_v, scores_g)
        v_rep = build_rep(v_bf)
        for m in range(M):
            softmax_unit(m, scores_v, scores_g, exps_v, exps_g, rwgate,
                         ccs, sums, recip, rw_recip)
            cm_v = ccs[:, m, :].unsqueeze(1).broadcast_to((BL, DV, S))
            outer_v = prods.tile([BL, DV, S], BF16, tag="prod2", name="ov", bufs=2)
            nc.vector.tensor_mul(outer_v, cm_v, v_rep[:, 0:DV, :])
            nc.sync.dma_start(out=memout_d[:, m, 0:DV, :], in_=outer_v)
            cm_g = ccs[:, m, :].unsqueeze(1).broadcast_to((BL, DG, S))
            outer_g = gprods.tile([BL, DG, S], BF16, tag="gprod2", name="og", bufs=2)
            nc.gpsimd.tensor_mul(outer_g, cm_g, v_rep[:, DV:D, :])
            nc.sync.dma_start(out=memout_d[:, m, DV:D, :], in_=outer_g)

    _split_excess_waits(nc)
    return nc


def _pack_inputs(x, h, memory_state, W_xr, W_hr, b_r, W_xz, W_hz, b_z,
                 W_xh, W_hh, b_h, Wr1, br1, Wr2, br2, Wq, bq, Wk, bk,
                 Wv, bv, Wg, bg):
    f32 = np.float32
    x = np.asarray(x, f32)
    h = np.asarray(h, f32)
    mem = np.asarray(memory_state, f32)

    # rin.T padded to 13 k-tiles with a ones row at 1536 (router/q bias)
    rint = np.zeros((KRIN * 128, B), f32)
    rint[0:IN] = x.T
    rint[IN:IN + H] = h.T
    rint[IN + H] = 1.0
    rint_bf = rint.reshape(KRIN, 128, B).astype(BF_NP)

    # xa weights: [x(512); ctx(64); bias; pad] -> 5 k-tiles
    def pack_wxa(Wf, b):
        out = np.zeros((KXA * 128, H), f32)
        out[0:IN + D] = np.asarray(Wf, f32)
        out[IN + D] = np.asarray(b, f32)
        return out.reshape(KXA, 128, H)
    wxa = np.stack([pack_wxa(W_xr, b_r), pack_wxa(W_xz, b_z),
                    pack_wxa(W_xh, b_h)]).astype(BF_NP)
    wh = np.stack([np.asarray(Wf, f32).reshape(KH, 128, H)
                   for Wf in (W_hr, W_hz, W_hh)]).astype(BF_NP)

    def pack_krin(Wf, b, n):
        out = np.zeros((KRIN * 128, n), f32)
        out[0:IN + H] = np.asarray(Wf, f32)
        out[IN + H] = np.asarray(b, f32)
        return out.reshape(KRIN, 128, n)
    wr1 = pack_krin(Wr1, br1, RH).astype(BF_NP)
    wq = pack_krin(Wq, bq, D).astype(BF_NP)

    wr2 = np.zeros((RH + 1, M), f32)
    wr2[0:RH] = np.asarray(Wr2, f32)
    wr2[RH] = np.asarray(br2, f32)
    wr2 = wr2.astype(BF_NP)

    wkvg = np.zeros((KKVG * 128, 2 * D + 1), f32)
    wkvg[0:H, 0:D] = np.asarray(Wk, f32)
    wkvg[0:H, D:2 * D] = np.asarray(Wv, f32)
    wkvg[0:H, 2 * D] = np.asarray(Wg, f32)[:, 0]
    wkvg[H, 0:D] = np.asarray(bk, f32)
    wkvg[H, D:2 * D] = np.asarray(bv, f32)
    wkvg[H, 2 * D] = np.asarray(bg, f32)[0]
    wkvg = wkvg.reshape(KKVG, 128, 2 * D + 1).astype(BF_NP)

    mem_t = np.ascontiguousarray(mem.transpose(0, 1, 3, 2))  # (B, M, D, S)
    mem16 = mem_t.astype(BF_NP)

    shared = dict(wxa=wxa, wh=wh, wr1=wr1, wq=wq, wr2=wr2, wkvg=wkvg)
    in_maps = []
    for c in range(NCORES):
        sl = slice(c * BL, (c + 1) * BL)
        in_maps.append(dict(
            rint=np.ascontiguousarray(rint_bf[:, :, sl]),
            h_b=np.ascontiguousarray(h[sl]),
            mem16=np.ascontiguousarray(mem16[sl]),
            **shared,
        ))
    return in_maps


def kernel(**inputs):
    if "nc" not in _CACHE:
        _CACHE["nc"] = build_bass()
    nc = _CACHE["nc"]
    in_maps = _pack_inputs(**inputs)
    res = run_bass_kernel_spmd(nc, in_maps, core_ids=list(range(NCORES)))
    h_new = np.concatenate([r["hout"] for r in res.results], axis=0)
    outer_t = np.concatenate([np.asarray(r["memout"], np.float32)
                              for r in res.results], axis=0)  # (B, M, D, S)
    mem_new = np.asarray(inputs["memory_state"], np.float32) \
        + outer_t.transpose(0, 1, 3, 2)
    return h_new, mem_new
